# revision 18
# baseline (speedup 1.0000x reference)
"""Trainium2 Bass kernel for a 4-layer GCN stack with dense batch-hop mixing.

Reference computation (N=32 graphs, M=2048 nodes, D=DOUT=128, E=32768 edges):
    Lx = sum_{i=0..3} gcn(Q_i x, W_i, b_i)
where Q_0 = I, Q_i = C_{i-1} @ ... @ C_0 (C = cached_adj hops over the n axis)
and gcn(h, W, b) = A (x)_m (h @ W) + b with A the (fixed) GCN normalized
adjacency operator acting on the node axis m.

Everything is linear and A / Q / W act on different axes, so they commute:
    Lx = A (x)_m [ sum_i (Q_i x) W_i ] + sum_i b_i
so the edge aggregation A is applied ONCE instead of 4 times.

Split of work:
  host   z_i = Q_i x      tiny [32,32] contraction -> one 128x262144 sgemm
  device Y[j,(l,e)] = sum_i z_i[l] @ W_i      (stage B, contraction (i,d)=512)
  device out[m,(l,e)] = sum_j A[m,j] Y[j,:]   (stage C, dense 2048x2048)

Sharding: data-parallel over n (4 graphs per core, 8 cores), no collectives.
Matmul operands are fp16 (full PE rate, 10-bit mantissa; measured rel err
~5e-4 vs fp32 reference).  PSUM accumulation is always fp32.

Host does only index preprocessing and the tiny Q contraction: degree/coef,
dense-A build, and layout packing so every DMA is contiguous per partition.
"""

import sys

import numpy as np

for _p in ("/opt/trn_rl_repo",):
    if _p not in sys.path:
        sys.path.insert(0, _p)

import concourse.bass as bass
import concourse.mybir as mybir
import concourse.tile as tile
from concourse import bacc
from concourse.bass_utils import run_bass_kernel_spmd

# Problem dims (hardcoded per contract).
N, M, D, DOUT, K, E = 32, 2048, 128, 128, 3, 32768
NCORES = 8
NL = N // NCORES          # graphs per core = 4
NI = K + 1                # layers = 4
JC = M // 128             # node-dim 128-chunks = 16
NE = NL * DOUT            # packed free dim = 512

# Matmul operand dtype: fp16 is full-rate on the PE and ~8x more accurate
# than bf16 (10-bit mantissa); fp32/fp32r run at 1/4 rate on TRN2.
DT_MODE = "fp16"
# Debug knobs: build only part of the pipeline / repeat it in-NEFF (timing).
STAGES = "all"
REPEAT = 1

LAST_RESULTS = None
_CACHED = {}

_DT = {
    "fp32": mybir.dt.float32,
    "fp32r": mybir.dt.float32r,
    "bf16": mybir.dt.bfloat16,
    "fp16": mybir.dt.float16,
}


def _np_dt(dt_mode):
    if dt_mode == "bf16":
        import ml_dtypes

        return ml_dtypes.bfloat16
    return {"fp16": np.float16, "fp32": np.float32, "fp32r": np.float32}[dt_mode]


def _build_nc(dt_mode: str, stages: str = "all", repeat: int = 1) -> bass.Bass:
    f32 = mybir.dt.float32
    io_dt = _DT[dt_mode]

    # Bacc (not raw Bass): its compile pipeline legalizes multi-wait
    # instructions into event-semaphore chains, which TRN2 requires.
    nc = bacc.Bacc(None, target_bir_lowering=False)
    # Host-packed layouts (p = SBUF partition index everywhere):
    #   zt [jc, p=d, q=(i*NL+l), f=j%128]   z_i[l]^T tiles
    #   w  [p=d, i, e]                      per-layer weights
    #   ad [mc, p=j%128, jc, f=m%128]       A^T tiles
    #   out [mc, p=m%128, l, e]
    zt_d = nc.dram_tensor("zt", [JC, 128, NI * NL, 128], io_dt, kind="ExternalInput")
    w_d = nc.dram_tensor("w", [128, NI, DOUT], io_dt, kind="ExternalInput")
    a_d = nc.dram_tensor("ad", [JC, 128, JC, 128], io_dt, kind="ExternalInput")
    o_d = nc.dram_tensor("out", [JC, 128, NL, DOUT], f32, kind="ExternalOutput")

    with tile.TileContext(nc) as tc:
        with (
            tc.tile_pool(name="const", bufs=1) as constp,
            tc.tile_pool(name="ztp", bufs=3) as ztp,
            tc.tile_pool(name="adp", bufs=3) as adp,
            tc.tile_pool(name="yp", bufs=1) as yp,
            tc.tile_pool(name="op", bufs=3) as op_,
            tc.tile_pool(name="ps_b", bufs=2, space="PSUM") as ps_b,
            tc.tile_pool(name="ps_c", bufs=3, space="PSUM") as ps_c,
            tc.tile_pool(name="ps_x", bufs=1, space="PSUM") as ps_x,
        ):
            w_sb = constp.tile([128, NI, DOUT], io_dt)
            nc.sync.dma_start(w_sb[:], w_d[:])
            y_sb = yp.tile([128, JC, NE], io_dt)

            # TRN2 instructions carry at most one semaphore wait.  A tiny
            # "touch" matmul into a scratch PSUM bank absorbs the DMA-
            # completion wait for each freshly loaded tile, so the real
            # matmuls never need more than one wait each.
            scratch = ps_x.tile([1, 2], f32)

            def touch(t3d):
                nc.tensor.matmul(
                    scratch[:],
                    lhsT=t3d[:, 0, 0:1],
                    rhs=t3d[:, 0, 0:2],
                    start=True,
                    stop=True,
                )

            touch(w_sb)

            for _rep in range(repeat):
                # Stage B: Y[j, l*128+e] = sum_i zt[(i,l)][d, j].T @ W[i][d, e]
                # All four l-groups accumulate into disjoint quarters of one
                # PSUM bank, drained by a single wide DVE copy.
                for jc in range(JC if stages in ("all", "ab") else 0):
                    zt_sb = ztp.tile([128, NI * NL, 128], io_dt, tag="zt")
                    nc.sync.dma_start(zt_sb[:], zt_d[jc])
                    touch(zt_sb)
                    ps = ps_b.tile([128, NL, DOUT], f32, tag="psb")
                    for l in range(NL):
                        for i in range(NI):
                            nc.tensor.matmul(
                                ps[:, l, :],
                                lhsT=zt_sb[:, i * NL + l, :],
                                rhs=w_sb[:, i, :],
                                start=(i == 0),
                                stop=(i == NI - 1),
                            )
                    nc.vector.tensor_copy(out=y_sb[:, jc, :], in_=ps[:])

                # Stage C: out[m, (l e)] += A^T[j, m].T @ Y[j, (l e)] over j.
                for mc in range(JC if stages in ("all", "c") else 0):
                    a_sb = adp.tile([128, JC, 128], io_dt, tag="ad")
                    nc.sync.dma_start(a_sb[:], a_d[mc])
                    touch(a_sb)
                    ps = ps_c.tile([128, NE], f32, tag="psc")
                    for jc in range(JC):
                        nc.tensor.matmul(
                            ps[:],
                            lhsT=a_sb[:, jc, :],
                            rhs=y_sb[:, jc, :],
                            start=(jc == 0),
                            stop=(jc == JC - 1),
                        )
                    o_sb = op_.tile([128, NE], f32, tag="ob")
                    nc.vector.tensor_copy(out=o_sb[:], in_=ps[:])
                    nc.sync.dma_start(o_d[mc], o_sb[:])

    nc.compile()
    return nc


def _get_nc(dt_mode: str) -> bass.Bass:
    key = (dt_mode, STAGES, REPEAT)
    if key not in _CACHED:
        _CACHED[key] = _build_nc(dt_mode, STAGES, REPEAT)
    return _CACHED[key]


def kernel(x, adj, cached_adj, Ws, bs, **_unused):
    global LAST_RESULTS
    x = np.asarray(x, dtype=np.float32)
    adj = np.asarray(adj, dtype=np.int64)
    cadj = np.asarray(cached_adj, dtype=np.float32)
    Ws = np.asarray(Ws, dtype=np.float32)
    bs = np.asarray(bs, dtype=np.float32)
    assert x.shape == (N, M, D) and adj.shape == (2, E)
    assert cadj.shape == (K, N, N) and Ws.shape == (NI, D, DOUT)

    io_np = _np_dt(DT_MODE)

    # ---- Dense normalized adjacency with self loops (host, index work only).
    src, dst = adj[0], adj[1]
    deg = np.bincount(dst, minlength=M).astype(np.float32) + 1.0
    dinv = 1.0 / np.sqrt(deg)
    coef = dinv[src] * dinv[dst]
    A = np.zeros((M, M), dtype=np.float32)
    np.add.at(A, (dst, src), coef)
    A[np.arange(M), np.arange(M)] += dinv * dinv
    # ad[mc, p, jc, f] = A^T[jc*128+p, mc*128+f] = A[mc*128+f, jc*128+p]
    ad = np.ascontiguousarray(
        A.reshape(JC, 128, JC, 128).transpose(0, 3, 2, 1), dtype=io_np
    )

    # ---- Hop prefixes Q_i and host contraction z_i = Q_i x (transposed).
    Qs = [np.eye(N, dtype=np.float32)]
    for i in range(K):
        Qs.append(cadj[i] @ Qs[-1])
    QQ = np.stack(Qs).reshape(NI * N, N)              # [(i n), n']
    xt_flat = x.transpose(0, 2, 1).reshape(N, D * M)  # [n', (d j)]
    Zt = (QQ @ xt_flat).reshape(NI, N, D, M)          # [i, n, d, j]

    w_dev = np.ascontiguousarray(Ws.transpose(1, 0, 2), dtype=io_np)  # [d, i, e]

    in_maps = []
    for c in range(NCORES):
        Zc = Zt[:, c * NL : (c + 1) * NL]             # [i, l, d, j]
        # zt[jc, p=d, q=i*NL+l, f] = Zc[i, l, d, jc*128+f]
        zt = np.ascontiguousarray(
            Zc.reshape(NI * NL, D, JC, 128).transpose(2, 1, 0, 3), dtype=io_np
        )
        in_maps.append({"zt": zt, "w": w_dev, "ad": ad})

    nc = _get_nc(DT_MODE)
    res = run_bass_kernel_spmd(nc, in_maps, core_ids=list(range(NCORES)))
    LAST_RESULTS = res

    # ---- Unshard: out[mc, p, l, e] -> [n, m, e].
    parts = [
        r["out"].transpose(2, 0, 1, 3).reshape(NL, M, DOUT) for r in res.results
    ]
    out = np.concatenate(parts, axis=0).astype(np.float32)

    bsum = bs.sum(axis=0)
    if np.any(bsum):
        out = out + bsum[None, None, :]
    return out
